# revision 22
# baseline (speedup 1.0000x reference)
"""Transformer block (pre-norm attn + MLP) on 8 NeuronCores, data-parallel over batch.

Full inputs in, full outputs out. Each core processes one batch element
x[i] : [1024, 768] through an identical Bass/Tile kernel.

Host-side exact refactoring:
  - LN gains fold into the following matmul weights: diag(g) @ W.
  - LN biases fold into: per-column bias on q/k (applied during psum->sbuf
    copy), b_proj_eff = b_proj + (b1 @ w_qkv_v) @ w_proj (softmax rows sum
    to one, so a v-bias passes through attention additively), and
    b_fc1_eff = b_fc1 + b2 @ w_fc1.
  - w_proj rows are re-laid-out head-aligned: block h occupies rows
    h*128+1 .. h*128+97 (row 0 pairs with the attention colsum row; zero).
    Row 97 of block 0 carries b_proj_eff (paired with a constant 1.0 row
    in o_fm), folding the proj bias into the matmul.
  - qkv / proj weights are cast to fp8e4 (TRN variant, max 240) on host;
    fc1/fc2 stay bf16. fp8 matmuls run in DoubleRow perf mode (2 k-tiles
    per pass). The residual stream, layernorm statistics and softmax
    normalization stay fp32.

On-chip dataflow (per core):
  LN1 (token-major, bn_stats, fp32 in -> fp8 out) -> PE transpose
    -> h_fm [C, N] fp8
  qkv (fp8 DoubleRow over kt pairs): q_fm/k_fm per-head feature-major bf16;
       v token-major with a leading ones column per head -> v_ext fp8
  attn per (nh, h), nh-major: S^T = k.T q bf16 (psum fp32, K=96) ->
       exp(s*scale - 2) on ACT -> E fp8; PV fp8 DoubleRow over token-tile
       pairs: o_unnorm[(1+96), n] = v_ext.T @ E (row 0 = colsum); colsum
       rows staged to cs[pair] via ACT; batched reciprocal (groups of 3)
       on DVE; gpsimd partition-broadcast; o = o_unnorm * rinv -> o_fm fp8
  proj (fp8 DoubleRow over head-block pairs, K=98 incl bias row):
       x1 = x + (o @ w_proj + b_proj), in place over x_tok  [one DVE add]
  LN2 -> PE transpose -> h2_fm bf16; MLP streamed over ff tiles (bf16):
       g = gelu(w_fc1.T h2 + b_fc1_eff) bf16; x2 += g.T w_fc2; + x1 + b_fc2
"""
import numpy as np
import ml_dtypes

import concourse.bass as bass
from concourse import bacc, mybir
from concourse.bass_utils import run_bass_kernel_spmd
from concourse.masks import make_identity
from concourse.tile import TileContext

P = 128
N = 1024          # tokens per core (batch element)
C = 768           # model dim
H = 8             # heads
DH = C // H       # 96
DFF = 4 * C       # 3072
NT = N // P       # 8 token tiles
KT = C // P       # 6 feature tiles
FFT = DFF // P    # 24 ff tiles
NH = 2            # halves of the token axis for attention
NC_ = N // NH     # 512
EPS = 1e-5
SCALE = DH ** -0.5
EXP_SHIFT = -2.0  # exp(s*scale + EXP_SHIFT): keeps E well inside fp8e4 range
VW = DH           # per-head v width (plus a leading ones column)
VS = VW + 2       # v head slot stride (98: keeps DoubleRow pair stride %16)
GROUP = 3         # softmax-normalization batch (psum-bank budget: 3)

F32 = mybir.dt.float32
BF16 = mybir.dt.bfloat16
F8 = mybir.dt.float8e4
F8E5 = mybir.dt.float8e5
DR = mybir.MatmulPerfMode.DoubleRow

_CACHED = {}


def build(taps=()):
    nc = bacc.Bacc("TRN2", debug=False)

    x_d = nc.dram_tensor("x", [N, C], F32, kind="ExternalInput")
    wqkv_d = nc.dram_tensor("w_qkv_e", [C, 3 * C], F8, kind="ExternalInput")
    wproj_d = nc.dram_tensor("w_proj_p", [H * P, C], F8, kind="ExternalInput")
    wfc1_d = nc.dram_tensor("w_fc1_e", [C, DFF], BF16, kind="ExternalInput")
    wfc2_d = nc.dram_tensor("w_fc2", [DFF, C], BF16, kind="ExternalInput")
    qkb_d = nc.dram_tensor("qk_bias", [P, 2 * H], F32, kind="ExternalInput")
    bf1_d = nc.dram_tensor("b_fc1_e", [DFF], F32, kind="ExternalInput")
    bf2_d = nc.dram_tensor("b_fc2", [C], F32, kind="ExternalInput")
    y_d = nc.dram_tensor("y", [N, C], F32, kind="ExternalOutput")

    tap_d = {}
    for name, shape, dt in [
        ("h_fm", [C, N], F8),
        ("q_fm", [H * P, N], BF16),
        ("k_fm", [H * P, N], BF16),
        ("v_ext", [N, H * VS], F8),
        ("o_fm", [H * P, N], F8),
        ("x1", [N, C], F32),
        ("h2_fm", [C, N], BF16),
    ]:
        if name in taps:
            tap_d[name] = nc.dram_tensor(
                "tap_" + name, shape, dt, kind="ExternalOutput"
            )

    def bcast_row(dram_t, width):
        return bass.AP(tensor=dram_t, offset=0, ap=[[0, P], [1, width]])

    with TileContext(nc) as tc:
        consts = tc.alloc_tile_pool(name="consts", bufs=1, side="left")
        pst = tc.alloc_tile_pool(name="pst", bufs=2, space="PSUM")

        # ---------- constants ----------
        identb = consts.tile([P, P], BF16)
        make_identity(nc, identb)
        eps_t = consts.tile([P, 1], F32)
        nc.vector.memset(eps_t, EPS)
        esh_t = consts.tile([P, 1], F32)
        nc.vector.memset(esh_t, EXP_SHIFT)
        scratch = consts.tile([P, 1], F32)
        nc.scalar.activation(
            out=scratch[:], in_=eps_t[:],
            func=mybir.ActivationFunctionType.Sqrt,
            bias=eps_t[:], scale=1.0,
        )
        qkb = consts.tile([P, 2 * H], F32)
        nc.gpsimd.dma_start(qkb[:], qkb_d[:, :])
        bf1c = consts.tile([P, FFT], F32)
        nc.gpsimd.dma_start(bf1c[:], bf1_d.rearrange("(t p) -> p t", p=P))

        # ---------- helpers ----------
        def layernorm_tile(x_ap, h_tile, lnt, stats):
            """h_tile[:] = (x_ap - mean) * rstd, cast to h_tile dtype."""
            st = stats.tile([P, 3, nc.vector.BN_STATS_DIM], F32, tag="bnst")
            for i in range(3):
                nc.vector.bn_stats(
                    out=st[:, i, :], in_=x_ap[:, i * 256:(i + 1) * 256]
                )
            mv = stats.tile([P, nc.vector.BN_AGGR_DIM], F32, tag="bnmv")
            nc.vector.bn_aggr(out=mv[:], in_=st[:])
            rstd = stats.tile([P, 1], F32, tag="rstd")
            nc.scalar.activation(
                out=rstd[:], in_=mv[:, 1:2],
                func=mybir.ActivationFunctionType.Sqrt,
                bias=eps_t[:], scale=1.0,
            )
            nc.vector.reciprocal(out=rstd[:], in_=rstd[:])
            nmr = stats.tile([P, 1], F32, tag="nmr")
            nc.vector.tensor_mul(nmr[:], mv[:, 0:1], rstd[:])
            nc.vector.tensor_scalar_mul(nmr[:], nmr[:], -1.0)
            nc.scalar.activation(
                out=h_tile[:], in_=x_ap[:],
                func=mybir.ActivationFunctionType.Identity,
                bias=nmr[:], scale=rstd[:],
            )

        def transpose_into(h_tile, dst_fm, nt, psump, idt, tpbufs=None):
            """dst_fm[:, kt, nt*P:(nt+1)*P] = h_tile[P, C].T via PE."""
            for kt in range(KT):
                tp = psump.tile([P, P], h_tile.dtype, tag="tp", bufs=tpbufs)
                nc.tensor.transpose(
                    tp[:], h_tile[:, kt * P:(kt + 1) * P], idt[:]
                )
                nc.vector.tensor_copy(
                    dst_fm[:, kt, nt * P:(nt + 1) * P], tp[:]
                )

        # ---------- phase 1-2: LN1 + transpose (h_fm in fp8) ----------
        # left-stack pools pushed in reverse release order
        xpool = tc.alloc_tile_pool(name="xpool", bufs=1, side="left")
        h2p = tc.alloc_tile_pool(name="h2p", bufs=1, side="left")
        opool = tc.alloc_tile_pool(name="opool", bufs=1, side="left")
        wprojp = tc.alloc_tile_pool(name="wprojp", bufs=1, side="left")
        hfmp = tc.alloc_tile_pool(name="hfmp", bufs=1, side="left")
        wqkvp = tc.alloc_tile_pool(name="wqkvp", bufs=1, side="left")
        wfc1p = tc.alloc_tile_pool(name="wfc1p", bufs=1, side="right")
        wfc2p = tc.alloc_tile_pool(name="wfc2p", bufs=1, side="right")

        x_tok = xpool.tile([P, NT, C], F32)
        for nt in range(NT):
            nc.sync.dma_start(x_tok[:, nt, :], x_d[nt * P:(nt + 1) * P, :])

        # weights on the same queue AFTER x, ordered by first-use time, so x
        # transfers get the full HBM bandwidth at startup
        wqkv = wqkvp.tile([P, KT, 3 * C], F8)
        for c0 in range(0, 3 * C, 1152):
            nc.sync.dma_start(
                wqkv[:, :, c0:c0 + 1152],
                wqkv_d.rearrange("(kt p) o -> p kt o", p=P)[:, :, c0:c0 + 1152],
            )
        wproj = wprojp.tile([P, H, C], F8)
        nc.sync.dma_start(wproj[:], wproj_d.rearrange("(hb p) c -> p hb c", p=P))
        wfc2 = wfc2p.tile([P, FFT, C], BF16)
        for f0 in range(0, FFT, 6):
            nc.sync.dma_start(
                wfc2[:, f0:f0 + 6, :],
                wfc2_d.rearrange("(ft p) c -> p ft c", p=P)[:, f0:f0 + 6, :],
            )
        wfc1 = wfc1p.tile([P, KT, DFF], BF16)
        for f0 in range(0, DFF, 1536):
            nc.sync.dma_start(
                wfc1[:, :, f0:f0 + 1536],
                wfc1_d.rearrange("(kt p) f -> p kt f", p=P)[:, :, f0:f0 + 1536],
            )

        lnt1 = tc.alloc_tile_pool(name="lnt1", bufs=3, side="left")
        stats1 = tc.alloc_tile_pool(name="stats1", bufs=4, side="left")
        h_fm = hfmp.tile([P, KT, N], F8)
        for nt in range(NT):
            h_t = lnt1.tile([P, C], BF16, tag="h1")
            layernorm_tile(x_tok[:, nt, :], h_t, lnt1, stats1)
            transpose_into(h_t, h_fm, nt, pst, identb)

        if "h_fm" in tap_d:
            nc.sync.dma_start(
                tap_d["h_fm"].rearrange("(kt p) n -> p kt n", p=P), h_fm[:]
            )
        stats1.release()
        lnt1.release()
        pst.release()

        # ---------- phase 3: qkv (fp8 DoubleRow over kt pairs) ----------
        qkvpool = tc.alloc_tile_pool(name="qkvpool", bufs=1, side="right")
        qkps = tc.alloc_tile_pool(name="qkps", bufs=3, space="PSUM")
        vps = tc.alloc_tile_pool(name="vps", bufs=3, space="PSUM")

        # v first (token-major, per-head slots [ones | v(96) | pad])
        v_ext = qkvpool.tile([P, NT, H, VS], F8)
        nc.gpsimd.memset(v_ext[:, :, :, 0], 1.0)
        for nt in range(NT):
            for half in range(2):  # 4 heads (384 cols) per psum
                pv = vps.tile([P, 4 * DH], F32, tag="v")
                c0 = 2 * C + half * 4 * DH
                for j in range(KT // 2):
                    nc.tensor.matmul(
                        pv[:],
                        h_fm[:, 2 * j:2 * j + 2, nt * P:(nt + 1) * P],
                        wqkv[:, 2 * j:2 * j + 2, c0:c0 + 4 * DH],
                        start=(j == 0), stop=(j == KT // 2 - 1),
                        perf_mode=DR,
                    )
                nc.scalar.activation(
                    out=v_ext[:, nt, half * 4:(half + 1) * 4, 1:VW + 1],
                    in_=pv.rearrange("p (h d) -> p h d", d=DH),
                    func=mybir.ActivationFunctionType.Copy,
                )

        q_fm = qkvpool.tile([P, H, N], BF16)
        k_fm = qkvpool.tile([P, H, N], BF16)

        if "q_fm" in tap_d:
            nc.sync.dma_start(
                tap_d["q_fm"].rearrange("(h p) n -> p h n", p=P), q_fm[:]
            )
        if "k_fm" in tap_d:
            nc.sync.dma_start(
                tap_d["k_fm"].rearrange("(h p) n -> p h n", p=P), k_fm[:]
            )
        if "v_ext" in tap_d:
            nc.sync.dma_start(
                tap_d["v_ext"].rearrange("(nt p) (h w) -> p nt h w", p=P, w=VS),
                v_ext[:],
            )

        vps.release()
        qkps.release()

        # ---------- phase 4: attention nh-major; proj interleaved ----
        epool = tc.alloc_tile_pool(name="epool", bufs=2, side="right")
        rrow = tc.alloc_tile_pool(name="rrow", bufs=GROUP, side="right")
        rbp = tc.alloc_tile_pool(name="rbp", bufs=GROUP, side="right")
        pops = tc.alloc_tile_pool(name="pops", bufs=2, space="PSUM")
        pqps = tc.alloc_tile_pool(name="pqps", bufs=2, space="PSUM")
        sps = tc.alloc_tile_pool(name="sps", bufs=2, space="PSUM")

        o_fm = opool.tile([P, H, N], F8)
        # bias row: o_fm[97, 0, :] = 1 pairs with w_proj_p row 97 (b_proj);
        # rows 97 of the other head blocks must be finite (paired w rows = 0).
        # Partition bases must be 32-aligned, so memset rows 96-97 (row 96 is
        # overwritten later by the normalization multiplies).
        nc.gpsimd.memset(o_fm[DH:DH + 2, :, :], 0.0)
        nc.gpsimd.memset(o_fm[DH:DH + 2, 0, :], 1.0)

        h2_fm = h2p.tile([P, KT, N], BF16)

        def emit_qk(h):
            for which, dst in ((0, q_fm), (1, k_fm)):
                col0 = which * C + h * DH
                for nh in range(NH):
                    pq = pqps.tile([P, NC_], F32, tag="PQ",
                                   name=f"pq_{h}_{which}_{nh}")
                    for j in range(KT // 2):
                        nc.tensor.matmul(
                            pq[:DH, :],
                            wqkv[:, 2 * j:2 * j + 2, col0:col0 + DH],
                            h_fm[:, 2 * j:2 * j + 2, nh * NC_:(nh + 1) * NC_],
                            start=(j == 0), stop=(j == KT // 2 - 1),
                            perf_mode=DR,
                        )
                    with tc.high_priority(64):
                        nc.vector.tensor_scalar_add(
                            dst[:DH, h, nh * NC_:(nh + 1) * NC_],
                            pq[:DH, :],
                            qkb[:DH, which * H + h:which * H + h + 1],
                        )

        def emit_scores(h, nh):
            e_t = epool.tile([P, NT, NC_], F8E5, tag="E", name=f"e_{h}_{nh}")
            for mt2 in range(NT // 2):
                ps_s = sps.tile([P, 2, NC_], F32, tag="S",
                                name=f"s_{h}_{nh}_{mt2}")
                for sub in range(2):
                    nc.tensor.matmul(
                        ps_s[:, sub, :],
                        k_fm[:DH, h, (2 * mt2 + sub) * P:(2 * mt2 + sub + 1) * P],
                        q_fm[:DH, h, nh * NC_:(nh + 1) * NC_],
                        start=True, stop=True,
                    )
                nc.scalar.activation(
                    out=e_t[:, 2 * mt2:2 * mt2 + 2, :], in_=ps_s[:],
                    func=mybir.ActivationFunctionType.Exp,
                    bias=esh_t[:], scale=SCALE,
                )
            return e_t

        def emit_pv(h, nh, e_t):
            po = pops.tile([P, NC_], F32, tag="PO", name=f"po_{h}_{nh}")
            for m in range(NT // 2):
                nc.tensor.matmul(
                    po[:VW + 1, :],
                    v_ext[:, 2 * m:2 * m + 2, h, 0:VW + 1],
                    e_t[:, 2 * m:2 * m + 2, :],
                    start=(m == 0), stop=(m == NT // 2 - 1),
                    perf_mode=DR,
                )
            # normalize: rinv on partition 0, broadcast, multiply into o_fm
            with tc.high_priority(64):
                rs = rrow.tile([1, NC_], F32, tag="rs", name=f"rs_{h}_{nh}")
                nc.vector.reciprocal(out=rs[0:1, :], in_=po[0:1, :])
                rb = rbp.tile([P, NC_], F32, tag="rb", name=f"rb_{h}_{nh}")
                nc.gpsimd.partition_broadcast(rb[:VW + 1, :], rs[0:1, :])
                nc.vector.tensor_mul(
                    o_fm[0:VW + 1, h, nh * NC_:(nh + 1) * NC_],
                    po[0:VW + 1, :], rb[0:VW + 1, :],
                )

        def emit_proj(nt):
            """x1[nt] = x[nt] + o @ w_proj + b_proj, in place over x_tok."""
            for c0, cw in ((0, 512), (512, 256)):
                pj = pops.tile([P, NC_], F32, tag="PO", name=f"pj_{nt}_{c0}")
                for b in range(H // 2):
                    nc.tensor.matmul(
                        pj[:, :cw],
                        o_fm[0:DH + 2, 2 * b:2 * b + 2, nt * P:(nt + 1) * P],
                        wproj[0:DH + 2, 2 * b:2 * b + 2, c0:c0 + cw],
                        start=(b == 0), stop=(b == H // 2 - 1),
                        perf_mode=DR,
                    )
                nc.vector.tensor_add(
                    x_tok[:, nt, c0:c0 + cw],
                    pj[:, :cw], x_tok[:, nt, c0:c0 + cw],
                )

        prev = None
        for nh in range(NH):
            for h in range(H):
                if nh == 0:
                    emit_qk(h)
                e_t = emit_scores(h, nh)
                if prev is not None:
                    emit_pv(*prev)
                prev = (h, nh, e_t)
            if nh == 0:
                wqkvp.release()
                hfmp.release()
            if prev is not None:
                emit_pv(*prev)
                prev = None
            # proj for this token half (overlaps next nh's attention)
            for nt in range(nh * NT // 2, (nh + 1) * NT // 2):
                emit_proj(nt)

        sps.release()
        pqps.release()
        rbp.release()
        rrow.release()
        epool.release()
        qkvpool.release()

        pst2 = tc.alloc_tile_pool(name="pst2", bufs=2, space="PSUM")
        lnt2 = tc.alloc_tile_pool(name="lnt2", bufs=3, side="left")
        stats2 = tc.alloc_tile_pool(name="stats2", bufs=4, side="left")
        for nt in range(NT):
            h_t = lnt2.tile([P, C], BF16, tag="h2")
            layernorm_tile(x_tok[:, nt, :], h_t, lnt2, stats2)
            transpose_into(h_t, h2_fm, nt, pst2, identb)

        if "o_fm" in tap_d:
            nc.sync.dma_start(
                tap_d["o_fm"].rearrange("(h p) n -> p h n", p=P), o_fm[:]
            )
        if "x1" in tap_d:
            nc.sync.dma_start(
                tap_d["x1"].rearrange("(nt p) c -> p nt c", p=P), x_tok[:]
            )
        if "h2_fm" in tap_d:
            nc.sync.dma_start(
                tap_d["h2_fm"].rearrange("(kt p) n -> p kt n", p=P), h2_fm[:]
            )

        stats2.release()
        lnt2.release()
        pst2.release()
        pops.release()

        wprojp.release()
        opool.release()

        # ---------- phase 8: MLP (bf16) ----------
        mlpc = tc.alloc_tile_pool(name="mlpc", bufs=1, side="left")
        bf2f = mlpc.tile([1, C], F32)
        nc.gpsimd.dma_start(
            bf2f[0:1, :], bass.AP(tensor=bf2_d, offset=0, ap=[[0, 1], [1, C]])
        )
        bf2s = mlpc.tile([1, C], BF16)
        nc.scalar.activation(
            out=bf2s[0:1, :], in_=bf2f[0:1, :],
            func=mybir.ActivationFunctionType.Copy,
        )
        ones1 = mlpc.tile([1, P], BF16)
        nc.vector.memset(ones1, 1.0)
        gpool = tc.alloc_tile_pool(name="gpool", bufs=3, side="left")
        outs = tc.alloc_tile_pool(name="outs", bufs=2, side="left")
        gps = tc.alloc_tile_pool(name="gps", bufs=3, space="PSUM")
        x2ps = tc.alloc_tile_pool(name="x2ps", bufs=2, space="PSUM")

        QW = 256  # token quarter width
        for q in range(4):
            pa = [x2ps.tile([P, 512], F32, tag="x2a", name=f"pa{q}_{jj}")
                  for jj in range(2)]
            pb = [x2ps.tile([P, 512], F32, tag="x2b", name=f"pb{q}_{jj}")
                  for jj in range(2)]
            for j in range(2):
                nc.tensor.matmul(
                    pa[j][:, 0:512], ones1[0:1, :], bf2s[0:1, 0:512],
                    start=True, stop=False,
                )
                nc.tensor.matmul(
                    pb[j][:, 0:256], ones1[0:1, :], bf2s[0:1, 512:768],
                    start=True, stop=False,
                )
            def emit_fc1(ff):
                pg = gps.tile([P, 512], F32, tag="G", name=f"pg_{q}_{ff}")
                for kt in range(KT):
                    nc.tensor.matmul(
                        pg[:, 0:QW],
                        wfc1[:, kt, ff * P:(ff + 1) * P],
                        h2_fm[:, kt, q * QW:(q + 1) * QW],
                        start=(kt == 0), stop=(kt == KT - 1),
                    )
                g_t = gpool.tile([P, QW], BF16, tag="g", name=f"g_{q}_{ff}")
                nc.scalar.activation(
                    out=g_t[:], in_=pg[:, 0:QW],
                    func=mybir.ActivationFunctionType.Gelu,
                    bias=bf1c[:, ff:ff + 1], scale=1.0,
                )
                return g_t

            def emit_fc2(ff, g_t):
                for j in range(2):
                    nc.tensor.matmul(
                        pa[j][:, 0:512],
                        g_t[:, j * P:(j + 1) * P],
                        wfc2[:, ff, 0:512],
                        start=False, stop=(ff == FFT - 1),
                    )
                    nc.tensor.matmul(
                        pb[j][:, 0:256],
                        g_t[:, j * P:(j + 1) * P],
                        wfc2[:, ff, 512:768],
                        start=False, stop=(ff == FFT - 1),
                    )

            g_prev = emit_fc1(0)
            for ff in range(FFT):
                if ff + 1 < FFT:
                    g_next = emit_fc1(ff + 1)
                else:
                    g_next = None
                emit_fc2(ff, g_prev)
                g_prev = g_next
            for j in range(2):
                nt = 2 * q + j
                o_t = outs.tile([P, C], F32, tag="y")
                nc.vector.tensor_add(
                    o_t[:, 0:512], pa[j][:, 0:512], x_tok[:, nt, 0:512]
                )
                nc.vector.tensor_add(
                    o_t[:, 512:768], pb[j][:, 0:256], x_tok[:, nt, 512:768]
                )
                nc.scalar.dma_start(y_d[nt * P:(nt + 1) * P, :], o_t[:])

        x2ps.release()
        gps.release()
        outs.release()
        gpool.release()
        mlpc.release()
        wfc2p.release()
        h2p.release()
        wfc1p.release()
        xpool.release()
        consts.release()

    nc.compile()
    return nc


def _f8(a):
    return np.ascontiguousarray(
        np.clip(a, -240.0, 240.0).astype(ml_dtypes.float8_e4m3)
    )


def _prep_inputs(inputs):
    """Host-side prep (exact refactoring of LN gains/biases into weights)."""
    f = lambda k: np.asarray(inputs[k], dtype=np.float32)
    x = f("x")
    w_qkv, w_proj, w_fc1, w_fc2 = f("w_qkv"), f("w_proj"), f("w_fc1"), f("w_fc2")
    ln1_g, ln1_b, ln2_g, ln2_b = f("ln1_g"), f("ln1_b"), f("ln2_g"), f("ln2_b")
    b_proj, b_fc1, b_fc2 = f("b_proj"), f("b_fc1"), f("b_fc2")

    bf = ml_dtypes.bfloat16
    w_qkv_e = ln1_g[:, None] * w_qkv
    qkv_bias = ln1_b @ w_qkv  # [2304]
    qk_bias = np.zeros((P, 2 * H), dtype=np.float32)
    for which in range(2):
        for h in range(H):
            qk_bias[0:DH, which * H + h] = qkv_bias[
                which * C + h * DH: which * C + (h + 1) * DH
            ]
    vb = qkv_bias[2 * C: 3 * C]  # v bias passes through softmax additively
    b_proj_e = b_proj + vb @ w_proj
    # head-aligned w_proj rows: block h rows 1..97 (row 0 pairs with colsum
    # row); row 97 of block 0 carries the proj bias (pairs with o_fm==1 row)
    w_proj_p = np.zeros((H * P, C), dtype=np.float32)
    for h in range(H):
        w_proj_p[h * P + 1: h * P + 1 + DH, :] = w_proj[h * DH:(h + 1) * DH, :]
    w_proj_p[DH + 1, :] = b_proj_e
    w_fc1_e = ln2_g[:, None] * w_fc1
    b_fc1_e = b_fc1 + ln2_b @ w_fc1

    common = {
        "w_qkv_e": _f8(w_qkv_e),
        "w_proj_p": _f8(w_proj_p),
        "w_fc1_e": np.ascontiguousarray(w_fc1_e.astype(bf)),
        "w_fc2": np.ascontiguousarray(w_fc2.astype(bf)),
        "qk_bias": qk_bias,
        "b_fc1_e": b_fc1_e,
        "b_fc2": b_fc2,
    }
    return [dict(common, x=np.ascontiguousarray(x[i])) for i in range(8)]


def kernel(**inputs):
    if "nc" not in _CACHED:
        _CACHED["nc"] = build()
    nc = _CACHED["nc"]
    in_maps = _prep_inputs(inputs)
    res = run_bass_kernel_spmd(nc, in_maps, core_ids=list(range(8)))
    out = np.stack([res.results[i]["y"] for i in range(8)], axis=0)
    return out.astype(np.float32)


# revision 23
# speedup vs baseline: 1.0067x; 1.0067x over previous
"""Transformer block (pre-norm attn + MLP) on 8 NeuronCores, data-parallel over batch.

Full inputs in, full outputs out. Each core processes one batch element
x[i] : [1024, 768] through an identical Bass/Tile kernel.

Host-side exact refactoring:
  - LN gains fold into the following matmul weights: diag(g) @ W.
  - LN biases fold into: per-column bias on q/k (applied during psum->sbuf
    copy), b_proj_eff = b_proj + (b1 @ w_qkv_v) @ w_proj (softmax rows sum
    to one, so a v-bias passes through attention additively), and
    b_fc1_eff = b_fc1 + b2 @ w_fc1.
  - w_proj rows are re-laid-out head-aligned: block h occupies rows
    h*128+1 .. h*128+97 (row 0 pairs with the attention colsum row; zero).
    Row 97 of block 0 carries b_proj_eff (paired with a constant 1.0 row
    in o_fm), folding the proj bias into the matmul.
  - qkv / proj weights are cast to fp8e4 (TRN variant, max 240) on host;
    fc1/fc2 stay bf16. fp8 matmuls run in DoubleRow perf mode (2 k-tiles
    per pass). The residual stream, layernorm statistics and softmax
    normalization stay fp32.

On-chip dataflow (per core):
  LN1 (token-major, bn_stats, fp32 in -> fp8 out) -> PE transpose
    -> h_fm [C, N] fp8
  qkv (fp8 DoubleRow over kt pairs): q_fm/k_fm per-head feature-major bf16;
       v token-major with a leading ones column per head -> v_ext fp8
  attn per (nh, h), nh-major: S^T = k.T q bf16 (psum fp32, K=96) ->
       exp(s*scale - 2) on ACT -> E fp8; PV fp8 DoubleRow over token-tile
       pairs: o_unnorm[(1+96), n] = v_ext.T @ E (row 0 = colsum); colsum
       rows staged to cs[pair] via ACT; batched reciprocal (groups of 3)
       on DVE; gpsimd partition-broadcast; o = o_unnorm * rinv -> o_fm fp8
  proj (fp8 DoubleRow over head-block pairs, K=98 incl bias row):
       x1 = x + (o @ w_proj + b_proj), in place over x_tok  [one DVE add]
  LN2 -> PE transpose -> h2_fm bf16; MLP streamed over ff tiles (bf16):
       g = gelu(w_fc1.T h2 + b_fc1_eff) bf16; x2 += g.T w_fc2; + x1 + b_fc2
"""
import numpy as np
import ml_dtypes

import concourse.bass as bass
from concourse import bacc, mybir
from concourse.bass_utils import run_bass_kernel_spmd
from concourse.masks import make_identity
from concourse.tile import TileContext

P = 128
N = 1024          # tokens per core (batch element)
C = 768           # model dim
H = 8             # heads
DH = C // H       # 96
DFF = 4 * C       # 3072
NT = N // P       # 8 token tiles
KT = C // P       # 6 feature tiles
FFT = DFF // P    # 24 ff tiles
NH = 2            # halves of the token axis for attention
NC_ = N // NH     # 512
EPS = 1e-5
SCALE = DH ** -0.5
EXP_SHIFT = -2.0  # exp(s*scale + EXP_SHIFT): keeps E well inside fp8e4 range
VW = DH           # per-head v width (plus a leading ones column)
VS = VW + 2       # v head slot stride (98: keeps DoubleRow pair stride %16)
GROUP = 3         # softmax-normalization batch (psum-bank budget: 3)

F32 = mybir.dt.float32
BF16 = mybir.dt.bfloat16
F8 = mybir.dt.float8e4
F8E5 = mybir.dt.float8e5
DR = mybir.MatmulPerfMode.DoubleRow

_CACHED = {}


def build(taps=()):
    nc = bacc.Bacc("TRN2", debug=False)

    x_d = nc.dram_tensor("x", [N, C], F32, kind="ExternalInput")
    wqkv_d = nc.dram_tensor("w_qkv_e", [C, 3 * C], F8, kind="ExternalInput")
    wproj_d = nc.dram_tensor("w_proj_p", [H * P, C], F8, kind="ExternalInput")
    wfc1_d = nc.dram_tensor("w_fc1_e", [C, DFF], BF16, kind="ExternalInput")
    wfc2_d = nc.dram_tensor("w_fc2", [DFF, C], BF16, kind="ExternalInput")
    qkb_d = nc.dram_tensor("qk_bias", [P, 2 * H], F32, kind="ExternalInput")
    bf1_d = nc.dram_tensor("b_fc1_e", [DFF], F32, kind="ExternalInput")
    bf2_d = nc.dram_tensor("b_fc2", [C], F32, kind="ExternalInput")
    y_d = nc.dram_tensor("y", [N, C], F32, kind="ExternalOutput")

    tap_d = {}
    for name, shape, dt in [
        ("h_fm", [C, N], F8),
        ("q_fm", [H * P, N], BF16),
        ("k_fm", [H * P, N], BF16),
        ("v_ext", [N, H * VS], F8),
        ("o_fm", [H * P, N], F8),
        ("x1", [N, C], F32),
        ("h2_fm", [C, N], BF16),
    ]:
        if name in taps:
            tap_d[name] = nc.dram_tensor(
                "tap_" + name, shape, dt, kind="ExternalOutput"
            )

    def bcast_row(dram_t, width):
        return bass.AP(tensor=dram_t, offset=0, ap=[[0, P], [1, width]])

    with TileContext(nc) as tc:
        consts = tc.alloc_tile_pool(name="consts", bufs=1, side="left")
        pst = tc.alloc_tile_pool(name="pst", bufs=2, space="PSUM")

        # ---------- constants ----------
        identb = consts.tile([P, P], BF16)
        make_identity(nc, identb)
        eps_t = consts.tile([P, 1], F32)
        nc.vector.memset(eps_t, EPS)
        esh_t = consts.tile([P, 1], F32)
        nc.vector.memset(esh_t, EXP_SHIFT)
        scratch = consts.tile([P, 1], F32)
        nc.scalar.activation(
            out=scratch[:], in_=eps_t[:],
            func=mybir.ActivationFunctionType.Sqrt,
            bias=eps_t[:], scale=1.0,
        )
        qkb = consts.tile([P, 2 * H], F32)
        nc.gpsimd.dma_start(qkb[:], qkb_d[:, :])
        bf1c = consts.tile([P, FFT], F32)
        nc.gpsimd.dma_start(bf1c[:], bf1_d.rearrange("(t p) -> p t", p=P))

        # ---------- helpers ----------
        def layernorm_tile(x_ap, h_tile, lnt, stats):
            """h_tile[:] = (x_ap - mean) * rstd, cast to h_tile dtype."""
            st = stats.tile([P, 3, nc.vector.BN_STATS_DIM], F32, tag="bnst")
            for i in range(3):
                nc.vector.bn_stats(
                    out=st[:, i, :], in_=x_ap[:, i * 256:(i + 1) * 256]
                )
            mv = stats.tile([P, nc.vector.BN_AGGR_DIM], F32, tag="bnmv")
            nc.vector.bn_aggr(out=mv[:], in_=st[:])
            rstd = stats.tile([P, 1], F32, tag="rstd")
            nc.scalar.activation(
                out=rstd[:], in_=mv[:, 1:2],
                func=mybir.ActivationFunctionType.Sqrt,
                bias=eps_t[:], scale=1.0,
            )
            nc.vector.reciprocal(out=rstd[:], in_=rstd[:])
            nmr = stats.tile([P, 1], F32, tag="nmr")
            nc.vector.tensor_mul(nmr[:], mv[:, 0:1], rstd[:])
            nc.vector.tensor_scalar_mul(nmr[:], nmr[:], -1.0)
            nc.scalar.activation(
                out=h_tile[:], in_=x_ap[:],
                func=mybir.ActivationFunctionType.Identity,
                bias=nmr[:], scale=rstd[:],
            )

        def transpose_into(h_tile, dst_fm, nt, psump, idt, tpbufs=None):
            """dst_fm[:, kt, nt*P:(nt+1)*P] = h_tile[P, C].T via PE.

            Drain copies alternate DVE/ACT: these regions are DVE-bound and
            the scalar engine has slack."""
            for kt in range(KT):
                tp = psump.tile([P, P], h_tile.dtype, tag="tp", bufs=tpbufs)
                nc.tensor.transpose(
                    tp[:], h_tile[:, kt * P:(kt + 1) * P], idt[:]
                )
                if kt % 2 == 0:
                    nc.vector.tensor_copy(
                        dst_fm[:, kt, nt * P:(nt + 1) * P], tp[:]
                    )
                else:
                    nc.scalar.activation(
                        out=dst_fm[:, kt, nt * P:(nt + 1) * P], in_=tp[:],
                        func=mybir.ActivationFunctionType.Copy,
                    )

        # ---------- phase 1-2: LN1 + transpose (h_fm in fp8) ----------
        # left-stack pools pushed in reverse release order
        xpool = tc.alloc_tile_pool(name="xpool", bufs=1, side="left")
        h2p = tc.alloc_tile_pool(name="h2p", bufs=1, side="left")
        opool = tc.alloc_tile_pool(name="opool", bufs=1, side="left")
        wprojp = tc.alloc_tile_pool(name="wprojp", bufs=1, side="left")
        hfmp = tc.alloc_tile_pool(name="hfmp", bufs=1, side="left")
        wqkvp = tc.alloc_tile_pool(name="wqkvp", bufs=1, side="left")
        wfc1p = tc.alloc_tile_pool(name="wfc1p", bufs=1, side="right")
        wfc2p = tc.alloc_tile_pool(name="wfc2p", bufs=1, side="right")

        x_tok = xpool.tile([P, NT, C], F32)
        for nt in range(NT):
            nc.sync.dma_start(x_tok[:, nt, :], x_d[nt * P:(nt + 1) * P, :])

        # weights on the same queue AFTER x, ordered by first-use time, so x
        # transfers get the full HBM bandwidth at startup
        wqkv = wqkvp.tile([P, KT, 3 * C], F8)
        for c0 in range(0, 3 * C, 1152):
            nc.sync.dma_start(
                wqkv[:, :, c0:c0 + 1152],
                wqkv_d.rearrange("(kt p) o -> p kt o", p=P)[:, :, c0:c0 + 1152],
            )
        wproj = wprojp.tile([P, H, C], F8)
        nc.sync.dma_start(wproj[:], wproj_d.rearrange("(hb p) c -> p hb c", p=P))
        wfc2 = wfc2p.tile([P, FFT, C], BF16)
        for f0 in range(0, FFT, 6):
            nc.sync.dma_start(
                wfc2[:, f0:f0 + 6, :],
                wfc2_d.rearrange("(ft p) c -> p ft c", p=P)[:, f0:f0 + 6, :],
            )
        wfc1 = wfc1p.tile([P, KT, DFF], BF16)
        for f0 in range(0, DFF, 1536):
            nc.sync.dma_start(
                wfc1[:, :, f0:f0 + 1536],
                wfc1_d.rearrange("(kt p) f -> p kt f", p=P)[:, :, f0:f0 + 1536],
            )

        lnt1 = tc.alloc_tile_pool(name="lnt1", bufs=3, side="left")
        stats1 = tc.alloc_tile_pool(name="stats1", bufs=4, side="left")
        h_fm = hfmp.tile([P, KT, N], F8)
        for nt in range(NT):
            h_t = lnt1.tile([P, C], BF16, tag="h1")
            layernorm_tile(x_tok[:, nt, :], h_t, lnt1, stats1)
            transpose_into(h_t, h_fm, nt, pst, identb)

        if "h_fm" in tap_d:
            nc.sync.dma_start(
                tap_d["h_fm"].rearrange("(kt p) n -> p kt n", p=P), h_fm[:]
            )
        stats1.release()
        lnt1.release()
        pst.release()

        # ---------- phase 3: qkv (fp8 DoubleRow over kt pairs) ----------
        qkvpool = tc.alloc_tile_pool(name="qkvpool", bufs=1, side="right")
        qkps = tc.alloc_tile_pool(name="qkps", bufs=3, space="PSUM")
        vps = tc.alloc_tile_pool(name="vps", bufs=3, space="PSUM")

        # v first (token-major, per-head slots [ones | v(96) | pad])
        v_ext = qkvpool.tile([P, NT, H, VS], F8)
        nc.gpsimd.memset(v_ext[:, :, :, 0], 1.0)
        for nt in range(NT):
            for half in range(2):  # 4 heads (384 cols) per psum
                pv = vps.tile([P, 4 * DH], F32, tag="v")
                c0 = 2 * C + half * 4 * DH
                for j in range(KT // 2):
                    nc.tensor.matmul(
                        pv[:],
                        h_fm[:, 2 * j:2 * j + 2, nt * P:(nt + 1) * P],
                        wqkv[:, 2 * j:2 * j + 2, c0:c0 + 4 * DH],
                        start=(j == 0), stop=(j == KT // 2 - 1),
                        perf_mode=DR,
                    )
                nc.scalar.activation(
                    out=v_ext[:, nt, half * 4:(half + 1) * 4, 1:VW + 1],
                    in_=pv.rearrange("p (h d) -> p h d", d=DH),
                    func=mybir.ActivationFunctionType.Copy,
                )

        q_fm = qkvpool.tile([P, H, N], BF16)
        k_fm = qkvpool.tile([P, H, N], BF16)

        if "q_fm" in tap_d:
            nc.sync.dma_start(
                tap_d["q_fm"].rearrange("(h p) n -> p h n", p=P), q_fm[:]
            )
        if "k_fm" in tap_d:
            nc.sync.dma_start(
                tap_d["k_fm"].rearrange("(h p) n -> p h n", p=P), k_fm[:]
            )
        if "v_ext" in tap_d:
            nc.sync.dma_start(
                tap_d["v_ext"].rearrange("(nt p) (h w) -> p nt h w", p=P, w=VS),
                v_ext[:],
            )

        vps.release()
        qkps.release()

        # ---------- phase 4: attention nh-major; proj interleaved ----
        epool = tc.alloc_tile_pool(name="epool", bufs=2, side="right")
        rrow = tc.alloc_tile_pool(name="rrow", bufs=GROUP, side="right")
        rbp = tc.alloc_tile_pool(name="rbp", bufs=GROUP, side="right")
        pops = tc.alloc_tile_pool(name="pops", bufs=2, space="PSUM")
        pqps = tc.alloc_tile_pool(name="pqps", bufs=2, space="PSUM")
        sps = tc.alloc_tile_pool(name="sps", bufs=2, space="PSUM")

        o_fm = opool.tile([P, H, N], F8)
        # bias row: o_fm[97, 0, :] = 1 pairs with w_proj_p row 97 (b_proj);
        # rows 97 of the other head blocks must be finite (paired w rows = 0).
        # Partition bases must be 32-aligned, so memset rows 96-97 (row 96 is
        # overwritten later by the normalization multiplies).
        nc.gpsimd.memset(o_fm[DH:DH + 2, :, :], 0.0)
        nc.gpsimd.memset(o_fm[DH:DH + 2, 0, :], 1.0)

        h2_fm = h2p.tile([P, KT, N], BF16)

        def emit_qk(h):
            for which, dst in ((0, q_fm), (1, k_fm)):
                col0 = which * C + h * DH
                for nh in range(NH):
                    pq = pqps.tile([P, NC_], F32, tag="PQ",
                                   name=f"pq_{h}_{which}_{nh}")
                    for j in range(KT // 2):
                        nc.tensor.matmul(
                            pq[:DH, :],
                            wqkv[:, 2 * j:2 * j + 2, col0:col0 + DH],
                            h_fm[:, 2 * j:2 * j + 2, nh * NC_:(nh + 1) * NC_],
                            start=(j == 0), stop=(j == KT // 2 - 1),
                            perf_mode=DR,
                        )
                    with tc.high_priority(64):
                        nc.vector.tensor_scalar_add(
                            dst[:DH, h, nh * NC_:(nh + 1) * NC_],
                            pq[:DH, :],
                            qkb[:DH, which * H + h:which * H + h + 1],
                        )

        def emit_scores(h, nh):
            e_t = epool.tile([P, NT, NC_], F8E5, tag="E", name=f"e_{h}_{nh}")
            for mt2 in range(NT // 2):
                ps_s = sps.tile([P, 2, NC_], F32, tag="S",
                                name=f"s_{h}_{nh}_{mt2}")
                for sub in range(2):
                    nc.tensor.matmul(
                        ps_s[:, sub, :],
                        k_fm[:DH, h, (2 * mt2 + sub) * P:(2 * mt2 + sub + 1) * P],
                        q_fm[:DH, h, nh * NC_:(nh + 1) * NC_],
                        start=True, stop=True,
                    )
                nc.scalar.activation(
                    out=e_t[:, 2 * mt2:2 * mt2 + 2, :], in_=ps_s[:],
                    func=mybir.ActivationFunctionType.Exp,
                    bias=esh_t[:], scale=SCALE,
                )
            return e_t

        def emit_pv(h, nh, e_t):
            po = pops.tile([P, NC_], F32, tag="PO", name=f"po_{h}_{nh}")
            for m in range(NT // 2):
                nc.tensor.matmul(
                    po[:VW + 1, :],
                    v_ext[:, 2 * m:2 * m + 2, h, 0:VW + 1],
                    e_t[:, 2 * m:2 * m + 2, :],
                    start=(m == 0), stop=(m == NT // 2 - 1),
                    perf_mode=DR,
                )
            # normalize: rinv on partition 0, broadcast, multiply into o_fm
            with tc.high_priority(64):
                rs = rrow.tile([1, NC_], F32, tag="rs", name=f"rs_{h}_{nh}")
                nc.vector.reciprocal(out=rs[0:1, :], in_=po[0:1, :])
                rb = rbp.tile([P, NC_], F32, tag="rb", name=f"rb_{h}_{nh}")
                nc.gpsimd.partition_broadcast(rb[:VW + 1, :], rs[0:1, :])
                nc.vector.tensor_mul(
                    o_fm[0:VW + 1, h, nh * NC_:(nh + 1) * NC_],
                    po[0:VW + 1, :], rb[0:VW + 1, :],
                )

        def emit_proj(nt):
            """x1[nt] = x[nt] + o @ w_proj + b_proj, in place over x_tok."""
            for c0, cw in ((0, 512), (512, 256)):
                pj = pops.tile([P, NC_], F32, tag="PO", name=f"pj_{nt}_{c0}")
                for b in range(H // 2):
                    nc.tensor.matmul(
                        pj[:, :cw],
                        o_fm[0:DH + 2, 2 * b:2 * b + 2, nt * P:(nt + 1) * P],
                        wproj[0:DH + 2, 2 * b:2 * b + 2, c0:c0 + cw],
                        start=(b == 0), stop=(b == H // 2 - 1),
                        perf_mode=DR,
                    )
                nc.vector.tensor_add(
                    x_tok[:, nt, c0:c0 + cw],
                    pj[:, :cw], x_tok[:, nt, c0:c0 + cw],
                )

        prev = None
        for nh in range(NH):
            for h in range(H):
                if nh == 0:
                    emit_qk(h)
                e_t = emit_scores(h, nh)
                if prev is not None:
                    emit_pv(*prev)
                prev = (h, nh, e_t)
            if nh == 0:
                wqkvp.release()
                hfmp.release()
            if prev is not None:
                emit_pv(*prev)
                prev = None
            # proj for this token half (overlaps next nh's attention)
            for nt in range(nh * NT // 2, (nh + 1) * NT // 2):
                emit_proj(nt)

        sps.release()
        pqps.release()
        rbp.release()
        rrow.release()
        epool.release()
        qkvpool.release()

        pst2 = tc.alloc_tile_pool(name="pst2", bufs=2, space="PSUM")
        lnt2 = tc.alloc_tile_pool(name="lnt2", bufs=3, side="left")
        stats2 = tc.alloc_tile_pool(name="stats2", bufs=4, side="left")
        for nt in range(NT):
            h_t = lnt2.tile([P, C], BF16, tag="h2")
            layernorm_tile(x_tok[:, nt, :], h_t, lnt2, stats2)
            transpose_into(h_t, h2_fm, nt, pst2, identb)

        if "o_fm" in tap_d:
            nc.sync.dma_start(
                tap_d["o_fm"].rearrange("(h p) n -> p h n", p=P), o_fm[:]
            )
        if "x1" in tap_d:
            nc.sync.dma_start(
                tap_d["x1"].rearrange("(nt p) c -> p nt c", p=P), x_tok[:]
            )
        if "h2_fm" in tap_d:
            nc.sync.dma_start(
                tap_d["h2_fm"].rearrange("(kt p) n -> p kt n", p=P), h2_fm[:]
            )

        stats2.release()
        lnt2.release()
        pst2.release()
        pops.release()

        wprojp.release()
        opool.release()

        # ---------- phase 8: MLP (bf16) ----------
        mlpc = tc.alloc_tile_pool(name="mlpc", bufs=1, side="left")
        bf2f = mlpc.tile([1, C], F32)
        nc.gpsimd.dma_start(
            bf2f[0:1, :], bass.AP(tensor=bf2_d, offset=0, ap=[[0, 1], [1, C]])
        )
        bf2s = mlpc.tile([1, C], BF16)
        nc.scalar.activation(
            out=bf2s[0:1, :], in_=bf2f[0:1, :],
            func=mybir.ActivationFunctionType.Copy,
        )
        ones1 = mlpc.tile([1, P], BF16)
        nc.vector.memset(ones1, 1.0)
        gpool = tc.alloc_tile_pool(name="gpool", bufs=3, side="left")
        outs = tc.alloc_tile_pool(name="outs", bufs=2, side="left")
        gps = tc.alloc_tile_pool(name="gps", bufs=3, space="PSUM")
        x2ps = tc.alloc_tile_pool(name="x2ps", bufs=2, space="PSUM")

        QW = 256  # token quarter width
        for q in range(4):
            pa = [x2ps.tile([P, 512], F32, tag="x2a", name=f"pa{q}_{jj}")
                  for jj in range(2)]
            pb = [x2ps.tile([P, 512], F32, tag="x2b", name=f"pb{q}_{jj}")
                  for jj in range(2)]
            for j in range(2):
                nc.tensor.matmul(
                    pa[j][:, 0:512], ones1[0:1, :], bf2s[0:1, 0:512],
                    start=True, stop=False,
                )
                nc.tensor.matmul(
                    pb[j][:, 0:256], ones1[0:1, :], bf2s[0:1, 512:768],
                    start=True, stop=False,
                )
            def emit_fc1(ff):
                pg = gps.tile([P, 512], F32, tag="G", name=f"pg_{q}_{ff}")
                for kt in range(KT):
                    nc.tensor.matmul(
                        pg[:, 0:QW],
                        wfc1[:, kt, ff * P:(ff + 1) * P],
                        h2_fm[:, kt, q * QW:(q + 1) * QW],
                        start=(kt == 0), stop=(kt == KT - 1),
                    )
                g_t = gpool.tile([P, QW], BF16, tag="g", name=f"g_{q}_{ff}")
                nc.scalar.activation(
                    out=g_t[:], in_=pg[:, 0:QW],
                    func=mybir.ActivationFunctionType.Gelu,
                    bias=bf1c[:, ff:ff + 1], scale=1.0,
                )
                return g_t

            def emit_fc2(ff, g_t):
                for j in range(2):
                    nc.tensor.matmul(
                        pa[j][:, 0:512],
                        g_t[:, j * P:(j + 1) * P],
                        wfc2[:, ff, 0:512],
                        start=False, stop=(ff == FFT - 1),
                    )
                    nc.tensor.matmul(
                        pb[j][:, 0:256],
                        g_t[:, j * P:(j + 1) * P],
                        wfc2[:, ff, 512:768],
                        start=False, stop=(ff == FFT - 1),
                    )

            g_prev = emit_fc1(0)
            for ff in range(FFT):
                if ff + 1 < FFT:
                    g_next = emit_fc1(ff + 1)
                else:
                    g_next = None
                emit_fc2(ff, g_prev)
                g_prev = g_next
            for j in range(2):
                nt = 2 * q + j
                o_t = outs.tile([P, C], F32, tag="y")
                nc.vector.tensor_add(
                    o_t[:, 0:512], pa[j][:, 0:512], x_tok[:, nt, 0:512]
                )
                nc.vector.tensor_add(
                    o_t[:, 512:768], pb[j][:, 0:256], x_tok[:, nt, 512:768]
                )
                nc.scalar.dma_start(y_d[nt * P:(nt + 1) * P, :], o_t[:])

        x2ps.release()
        gps.release()
        outs.release()
        gpool.release()
        mlpc.release()
        wfc2p.release()
        h2p.release()
        wfc1p.release()
        xpool.release()
        consts.release()

    nc.compile()
    return nc


def _f8(a):
    return np.ascontiguousarray(
        np.clip(a, -240.0, 240.0).astype(ml_dtypes.float8_e4m3)
    )


def _prep_inputs(inputs):
    """Host-side prep (exact refactoring of LN gains/biases into weights)."""
    f = lambda k: np.asarray(inputs[k], dtype=np.float32)
    x = f("x")
    w_qkv, w_proj, w_fc1, w_fc2 = f("w_qkv"), f("w_proj"), f("w_fc1"), f("w_fc2")
    ln1_g, ln1_b, ln2_g, ln2_b = f("ln1_g"), f("ln1_b"), f("ln2_g"), f("ln2_b")
    b_proj, b_fc1, b_fc2 = f("b_proj"), f("b_fc1"), f("b_fc2")

    bf = ml_dtypes.bfloat16
    w_qkv_e = ln1_g[:, None] * w_qkv
    qkv_bias = ln1_b @ w_qkv  # [2304]
    qk_bias = np.zeros((P, 2 * H), dtype=np.float32)
    for which in range(2):
        for h in range(H):
            qk_bias[0:DH, which * H + h] = qkv_bias[
                which * C + h * DH: which * C + (h + 1) * DH
            ]
    vb = qkv_bias[2 * C: 3 * C]  # v bias passes through softmax additively
    b_proj_e = b_proj + vb @ w_proj
    # head-aligned w_proj rows: block h rows 1..97 (row 0 pairs with colsum
    # row); row 97 of block 0 carries the proj bias (pairs with o_fm==1 row)
    w_proj_p = np.zeros((H * P, C), dtype=np.float32)
    for h in range(H):
        w_proj_p[h * P + 1: h * P + 1 + DH, :] = w_proj[h * DH:(h + 1) * DH, :]
    w_proj_p[DH + 1, :] = b_proj_e
    w_fc1_e = ln2_g[:, None] * w_fc1
    b_fc1_e = b_fc1 + ln2_b @ w_fc1

    common = {
        "w_qkv_e": _f8(w_qkv_e),
        "w_proj_p": _f8(w_proj_p),
        "w_fc1_e": np.ascontiguousarray(w_fc1_e.astype(bf)),
        "w_fc2": np.ascontiguousarray(w_fc2.astype(bf)),
        "qk_bias": qk_bias,
        "b_fc1_e": b_fc1_e,
        "b_fc2": b_fc2,
    }
    return [dict(common, x=np.ascontiguousarray(x[i])) for i in range(8)]


def kernel(**inputs):
    if "nc" not in _CACHED:
        _CACHED["nc"] = build()
    nc = _CACHED["nc"]
    in_maps = _prep_inputs(inputs)
    res = run_bass_kernel_spmd(nc, in_maps, core_ids=list(range(8)))
    out = np.stack([res.results[i]["y"] for i in range(8)], axis=0)
    return out.astype(np.float32)


# revision 24
# speedup vs baseline: 1.2081x; 1.2000x over previous
"""Transformer block (pre-norm attn + MLP) on 8 NeuronCores, data-parallel over batch.

Full inputs in, full outputs out. Each core processes one batch element
x[i] : [1024, 768] through an identical Bass/Tile kernel.

Host-side exact refactoring:
  - LN gains fold into the following matmul weights: diag(g) @ W.
  - LN biases fold into: per-column bias on q/k (applied during psum->sbuf
    copy), b_proj_eff = b_proj + (b1 @ w_qkv_v) @ w_proj (softmax rows sum
    to one, so a v-bias passes through attention additively), and
    b_fc1_eff = b_fc1 + b2 @ w_fc1.
  - w_proj rows are re-laid-out head-aligned: block h occupies rows
    h*128+1 .. h*128+97 (row 0 pairs with the attention colsum row; zero).
    Row 97 of block 0 carries b_proj_eff (paired with a constant 1.0 row
    in o_fm), folding the proj bias into the matmul.
  - qkv / proj weights are cast to fp8e4 (TRN variant, max 240) on host;
    fc1/fc2 stay bf16. fp8 matmuls run in DoubleRow perf mode (2 k-tiles
    per pass). The residual stream, layernorm statistics and softmax
    normalization stay fp32.

On-chip dataflow (per core):
  LN1 (token-major, bn_stats, fp32 in -> fp8 out) -> PE transpose
    -> h_fm [C, N] fp8
  qkv (fp8 DoubleRow over kt pairs): q_fm/k_fm per-head feature-major bf16;
       v token-major with a leading ones column per head -> v_ext fp8
  attn per (nh, h), nh-major: S^T = k.T q bf16 (psum fp32, K=96) ->
       exp(s*scale - 2) on ACT -> E fp8; PV fp8 DoubleRow over token-tile
       pairs: o_unnorm[(1+96), n] = v_ext.T @ E (row 0 = colsum); colsum
       rows staged to cs[pair] via ACT; batched reciprocal (groups of 3)
       on DVE; gpsimd partition-broadcast; o = o_unnorm * rinv -> o_fm fp8
  proj (fp8 DoubleRow over head-block pairs, K=98 incl bias row):
       x1 = x + (o @ w_proj + b_proj), in place over x_tok  [one DVE add]
  LN2 -> PE transpose -> h2_fm bf16; MLP streamed over ff tiles (bf16):
       g = gelu(w_fc1.T h2 + b_fc1_eff) bf16; x2 += g.T w_fc2; + x1 + b_fc2
"""
import numpy as np
import ml_dtypes

import concourse.bass as bass
from concourse import bacc, mybir
from concourse.bass_utils import run_bass_kernel_spmd
from concourse.masks import make_identity
from concourse.tile import TileContext

P = 128
N = 1024          # tokens per core (batch element)
C = 768           # model dim
H = 8             # heads
DH = C // H       # 96
DFF = 4 * C       # 3072
NT = N // P       # 8 token tiles
KT = C // P       # 6 feature tiles
FFT = DFF // P    # 24 ff tiles
NH = 2            # halves of the token axis for attention
NC_ = N // NH     # 512
EPS = 1e-5
SCALE = DH ** -0.5
EXP_SHIFT = -2.0  # exp(s*scale + EXP_SHIFT): keeps E well inside fp8e4 range
VW = DH           # per-head v width (plus a leading ones column)
VS = VW + 2       # v head slot stride (98: keeps DoubleRow pair stride %16)
GROUP = 3         # softmax-normalization batch (psum-bank budget: 3)

F32 = mybir.dt.float32
BF16 = mybir.dt.bfloat16
F8 = mybir.dt.float8e4
F8E5 = mybir.dt.float8e5
DR = mybir.MatmulPerfMode.DoubleRow

_CACHED = {}


def build(taps=()):
    nc = bacc.Bacc("TRN2", debug=False)

    x_d = nc.dram_tensor("x", [N, C], F32, kind="ExternalInput")
    wqkv_d = nc.dram_tensor("w_qkv_e", [C, 3 * C], F8, kind="ExternalInput")
    wproj_d = nc.dram_tensor("w_proj_p", [H * P, C], F8, kind="ExternalInput")
    wfc1_d = nc.dram_tensor("w_fc1_e", [C, DFF], BF16, kind="ExternalInput")
    wfc2_d = nc.dram_tensor("w_fc2", [DFF, C], BF16, kind="ExternalInput")
    qkb_d = nc.dram_tensor("qk_bias", [P, 2 * H], F32, kind="ExternalInput")
    bf1_d = nc.dram_tensor("b_fc1_e", [DFF], F32, kind="ExternalInput")
    bf2_d = nc.dram_tensor("b_fc2", [C], F32, kind="ExternalInput")
    y_d = nc.dram_tensor("y", [N, C], F32, kind="ExternalOutput")

    tap_d = {}
    for name, shape, dt in [
        ("h_fm", [C, N], F8),
        ("q_fm", [H * P, N], BF16),
        ("k_fm", [H * P, N], BF16),
        ("v_ext", [N, H * VS], F8),
        ("o_fm", [H * P, N], F8),
        ("x1", [N, C], F32),
        ("h2_fm", [C, N], BF16),
    ]:
        if name in taps:
            tap_d[name] = nc.dram_tensor(
                "tap_" + name, shape, dt, kind="ExternalOutput"
            )

    def bcast_row(dram_t, width):
        return bass.AP(tensor=dram_t, offset=0, ap=[[0, P], [1, width]])

    with TileContext(nc) as tc:
        consts = tc.alloc_tile_pool(name="consts", bufs=1, side="left")
        pst = tc.alloc_tile_pool(name="pst", bufs=4, space="PSUM")

        # ---------- constants ----------
        identb = consts.tile([P, P], BF16)
        make_identity(nc, identb)
        eps_t = consts.tile([P, 1], F32)
        nc.vector.memset(eps_t, EPS)
        esh_t = consts.tile([P, 1], F32)
        nc.vector.memset(esh_t, EXP_SHIFT)
        scratch = consts.tile([P, 1], F32)
        nc.scalar.activation(
            out=scratch[:], in_=eps_t[:],
            func=mybir.ActivationFunctionType.Sqrt,
            bias=eps_t[:], scale=1.0,
        )
        qkb = consts.tile([P, 2 * H], F32)
        nc.gpsimd.dma_start(qkb[:], qkb_d[:, :])
        bf1c = consts.tile([P, FFT], F32)
        nc.gpsimd.dma_start(bf1c[:], bf1_d.rearrange("(t p) -> p t", p=P))

        # ---------- helpers ----------
        def layernorm_tile(x_ap, h_tile, lnt, stats):
            """h_tile[:] = (x_ap - mean) * rstd, cast to h_tile dtype."""
            st = stats.tile([P, 3, nc.vector.BN_STATS_DIM], F32, tag="bnst")
            for i in range(3):
                nc.vector.bn_stats(
                    out=st[:, i, :], in_=x_ap[:, i * 256:(i + 1) * 256]
                )
            mv = stats.tile([P, nc.vector.BN_AGGR_DIM], F32, tag="bnmv")
            nc.vector.bn_aggr(out=mv[:], in_=st[:])
            rstd = stats.tile([P, 1], F32, tag="rstd")
            nc.scalar.activation(
                out=rstd[:], in_=mv[:, 1:2],
                func=mybir.ActivationFunctionType.Sqrt,
                bias=eps_t[:], scale=1.0,
            )
            nc.vector.reciprocal(out=rstd[:], in_=rstd[:])
            nmr = stats.tile([P, 1], F32, tag="nmr")
            nc.vector.tensor_mul(nmr[:], mv[:, 0:1], rstd[:])
            nc.vector.tensor_scalar_mul(nmr[:], nmr[:], -1.0)
            nc.scalar.activation(
                out=h_tile[:], in_=x_ap[:],
                func=mybir.ActivationFunctionType.Identity,
                bias=nmr[:], scale=rstd[:],
            )

        def transpose_into(h_tile, dst_fm, nt, psump, idt, tpbufs=None):
            """dst_fm[:, kt, nt*P:(nt+1)*P] = h_tile[P, C].T via PE.

            Drain copies alternate DVE/ACT: these regions are DVE-bound and
            the scalar engine has slack."""
            for kt in range(KT):
                tp = psump.tile([P, P], h_tile.dtype, tag="tp", bufs=tpbufs)
                nc.tensor.transpose(
                    tp[:], h_tile[:, kt * P:(kt + 1) * P], idt[:]
                )
                if kt % 2 == 0:
                    nc.vector.tensor_copy(
                        dst_fm[:, kt, nt * P:(nt + 1) * P], tp[:]
                    )
                else:
                    nc.scalar.activation(
                        out=dst_fm[:, kt, nt * P:(nt + 1) * P], in_=tp[:],
                        func=mybir.ActivationFunctionType.Copy,
                    )

        # ---------- phase 1-2: LN1 + transpose (h_fm in fp8) ----------
        # left-stack pools pushed in reverse release order
        xpool = tc.alloc_tile_pool(name="xpool", bufs=1, side="left")
        h2p = tc.alloc_tile_pool(name="h2p", bufs=1, side="left")
        opool = tc.alloc_tile_pool(name="opool", bufs=1, side="left")
        wprojp = tc.alloc_tile_pool(name="wprojp", bufs=1, side="left")
        hfmp = tc.alloc_tile_pool(name="hfmp", bufs=1, side="left")
        wqkvp = tc.alloc_tile_pool(name="wqkvp", bufs=1, side="left")
        wfc1p = tc.alloc_tile_pool(name="wfc1p", bufs=1, side="right")
        wfc2p = tc.alloc_tile_pool(name="wfc2p", bufs=1, side="right")

        x_tok = xpool.tile([P, NT, C], F32)
        for nt in range(NT):
            nc.sync.dma_start(x_tok[:, nt, :], x_d[nt * P:(nt + 1) * P, :])

        # weights on the same queue AFTER x, ordered by first-use time, so x
        # transfers get the full HBM bandwidth at startup
        wqkv = wqkvp.tile([P, KT, 3 * C], F8)
        for c0 in range(0, 3 * C, 1152):
            nc.sync.dma_start(
                wqkv[:, :, c0:c0 + 1152],
                wqkv_d.rearrange("(kt p) o -> p kt o", p=P)[:, :, c0:c0 + 1152],
            )
        wproj = wprojp.tile([P, H, C], F8)
        nc.sync.dma_start(wproj[:], wproj_d.rearrange("(hb p) c -> p hb c", p=P))
        wfc2 = wfc2p.tile([P, FFT, C], BF16)
        for f0 in range(0, FFT, 6):
            nc.sync.dma_start(
                wfc2[:, f0:f0 + 6, :],
                wfc2_d.rearrange("(ft p) c -> p ft c", p=P)[:, f0:f0 + 6, :],
            )
        wfc1 = wfc1p.tile([P, KT, DFF], BF16)
        for f0 in range(0, DFF, 1536):
            nc.sync.dma_start(
                wfc1[:, :, f0:f0 + 1536],
                wfc1_d.rearrange("(kt p) f -> p kt f", p=P)[:, :, f0:f0 + 1536],
            )

        lnt1 = tc.alloc_tile_pool(name="lnt1", bufs=4, side="left")
        stats1 = tc.alloc_tile_pool(name="stats1", bufs=6, side="left")
        h_fm = hfmp.tile([P, KT, N], F8)
        for nt in range(NT):
            h_t = lnt1.tile([P, C], BF16, tag="h1")
            layernorm_tile(x_tok[:, nt, :], h_t, lnt1, stats1)
            transpose_into(h_t, h_fm, nt, pst, identb)

        if "h_fm" in tap_d:
            nc.sync.dma_start(
                tap_d["h_fm"].rearrange("(kt p) n -> p kt n", p=P), h_fm[:]
            )
        stats1.release()
        lnt1.release()
        pst.release()

        # ---------- phase 3: qkv (fp8 DoubleRow over kt pairs) ----------
        qkvpool = tc.alloc_tile_pool(name="qkvpool", bufs=1, side="right")
        qkps = tc.alloc_tile_pool(name="qkps", bufs=3, space="PSUM")
        vps = tc.alloc_tile_pool(name="vps", bufs=4, space="PSUM")

        # v first (token-major, per-head slots [ones | v(96) | pad])
        v_ext = qkvpool.tile([P, NT, H, VS], F8)
        nc.gpsimd.memset(v_ext[:, :, :, 0], 1.0)
        for nt in range(NT):
            for half in range(2):  # 4 heads (384 cols) per psum
                pv = vps.tile([P, 4 * DH], F32, tag="v")
                c0 = 2 * C + half * 4 * DH
                for j in range(KT // 2):
                    nc.tensor.matmul(
                        pv[:],
                        h_fm[:, 2 * j:2 * j + 2, nt * P:(nt + 1) * P],
                        wqkv[:, 2 * j:2 * j + 2, c0:c0 + 4 * DH],
                        start=(j == 0), stop=(j == KT // 2 - 1),
                        perf_mode=DR,
                    )
                nc.scalar.activation(
                    out=v_ext[:, nt, half * 4:(half + 1) * 4, 1:VW + 1],
                    in_=pv.rearrange("p (h d) -> p h d", d=DH),
                    func=mybir.ActivationFunctionType.Copy,
                )

        q_fm = qkvpool.tile([P, H, N], BF16)
        k_fm = qkvpool.tile([P, H, N], BF16)

        if "q_fm" in tap_d:
            nc.sync.dma_start(
                tap_d["q_fm"].rearrange("(h p) n -> p h n", p=P), q_fm[:]
            )
        if "k_fm" in tap_d:
            nc.sync.dma_start(
                tap_d["k_fm"].rearrange("(h p) n -> p h n", p=P), k_fm[:]
            )
        if "v_ext" in tap_d:
            nc.sync.dma_start(
                tap_d["v_ext"].rearrange("(nt p) (h w) -> p nt h w", p=P, w=VS),
                v_ext[:],
            )

        vps.release()
        qkps.release()

        # ---------- phase 4: attention nh-major; proj interleaved ----
        epool = tc.alloc_tile_pool(name="epool", bufs=2, side="right")
        rrow = tc.alloc_tile_pool(name="rrow", bufs=GROUP, side="right")
        rbp = tc.alloc_tile_pool(name="rbp", bufs=GROUP, side="right")
        pops = tc.alloc_tile_pool(name="pops", bufs=2, space="PSUM")
        pqps = tc.alloc_tile_pool(name="pqps", bufs=2, space="PSUM")
        sps = tc.alloc_tile_pool(name="sps", bufs=2, space="PSUM")

        o_fm = opool.tile([P, H, N], F8)
        # bias row: o_fm[97, 0, :] = 1 pairs with w_proj_p row 97 (b_proj);
        # rows 97 of the other head blocks must be finite (paired w rows = 0).
        # Partition bases must be 32-aligned, so memset rows 96-97 (row 96 is
        # overwritten later by the normalization multiplies).
        nc.gpsimd.memset(o_fm[DH:DH + 2, :, :], 0.0)
        nc.gpsimd.memset(o_fm[DH:DH + 2, 0, :], 1.0)

        h2_fm = h2p.tile([P, KT, N], BF16)

        def emit_qk(h):
            for which, dst in ((0, q_fm), (1, k_fm)):
                col0 = which * C + h * DH
                for nh in range(NH):
                    pq = pqps.tile([P, NC_], F32, tag="PQ",
                                   name=f"pq_{h}_{which}_{nh}")
                    for j in range(KT // 2):
                        nc.tensor.matmul(
                            pq[:DH, :],
                            wqkv[:, 2 * j:2 * j + 2, col0:col0 + DH],
                            h_fm[:, 2 * j:2 * j + 2, nh * NC_:(nh + 1) * NC_],
                            start=(j == 0), stop=(j == KT // 2 - 1),
                            perf_mode=DR,
                        )
                    with tc.high_priority(64):
                        nc.vector.tensor_scalar_add(
                            dst[:DH, h, nh * NC_:(nh + 1) * NC_],
                            pq[:DH, :],
                            qkb[:DH, which * H + h:which * H + h + 1],
                        )

        def emit_scores(h, nh):
            e_t = epool.tile([P, NT, NC_], F8E5, tag="E", name=f"e_{h}_{nh}")
            for mt2 in range(NT // 2):
                ps_s = sps.tile([P, 2, NC_], F32, tag="S",
                                name=f"s_{h}_{nh}_{mt2}")
                for sub in range(2):
                    nc.tensor.matmul(
                        ps_s[:, sub, :],
                        k_fm[:DH, h, (2 * mt2 + sub) * P:(2 * mt2 + sub + 1) * P],
                        q_fm[:DH, h, nh * NC_:(nh + 1) * NC_],
                        start=True, stop=True,
                    )
                nc.scalar.activation(
                    out=e_t[:, 2 * mt2:2 * mt2 + 2, :], in_=ps_s[:],
                    func=mybir.ActivationFunctionType.Exp,
                    bias=esh_t[:], scale=SCALE,
                )
            return e_t

        def emit_pv(h, nh, e_t):
            po = pops.tile([P, NC_], F32, tag="PO", name=f"po_{h}_{nh}")
            for m in range(NT // 2):
                nc.tensor.matmul(
                    po[:VW + 1, :],
                    v_ext[:, 2 * m:2 * m + 2, h, 0:VW + 1],
                    e_t[:, 2 * m:2 * m + 2, :],
                    start=(m == 0), stop=(m == NT // 2 - 1),
                    perf_mode=DR,
                )
            # normalize: rinv on partition 0, broadcast, multiply into o_fm
            with tc.high_priority(64):
                rs = rrow.tile([1, NC_], F32, tag="rs", name=f"rs_{h}_{nh}")
                nc.vector.reciprocal(out=rs[0:1, :], in_=po[0:1, :])
                rb = rbp.tile([P, NC_], F32, tag="rb", name=f"rb_{h}_{nh}")
                nc.gpsimd.partition_broadcast(rb[:VW + 1, :], rs[0:1, :])
                nc.vector.tensor_mul(
                    o_fm[0:VW + 1, h, nh * NC_:(nh + 1) * NC_],
                    po[0:VW + 1, :], rb[0:VW + 1, :],
                )

        def emit_proj(nt):
            """x1[nt] = x[nt] + o @ w_proj + b_proj, in place over x_tok."""
            for c0, cw in ((0, 512), (512, 256)):
                pj = pops.tile([P, NC_], F32, tag="PO", name=f"pj_{nt}_{c0}")
                for b in range(H // 2):
                    nc.tensor.matmul(
                        pj[:, :cw],
                        o_fm[0:DH + 2, 2 * b:2 * b + 2, nt * P:(nt + 1) * P],
                        wproj[0:DH + 2, 2 * b:2 * b + 2, c0:c0 + cw],
                        start=(b == 0), stop=(b == H // 2 - 1),
                        perf_mode=DR,
                    )
                nc.vector.tensor_add(
                    x_tok[:, nt, c0:c0 + cw],
                    pj[:, :cw], x_tok[:, nt, c0:c0 + cw],
                )

        prev = None
        for nh in range(NH):
            for h in range(H):
                if nh == 0:
                    emit_qk(h)
                e_t = emit_scores(h, nh)
                if prev is not None:
                    emit_pv(*prev)
                prev = (h, nh, e_t)
            if nh == 0:
                wqkvp.release()
                hfmp.release()
            if prev is not None:
                emit_pv(*prev)
                prev = None
            # proj for this token half (overlaps next nh's attention)
            for nt in range(nh * NT // 2, (nh + 1) * NT // 2):
                emit_proj(nt)

        sps.release()
        pqps.release()
        rbp.release()
        rrow.release()
        epool.release()
        qkvpool.release()

        pst2 = tc.alloc_tile_pool(name="pst2", bufs=4, space="PSUM")
        lnt2 = tc.alloc_tile_pool(name="lnt2", bufs=3, side="left")
        stats2 = tc.alloc_tile_pool(name="stats2", bufs=6, side="left")
        for nt in range(NT):
            h_t = lnt2.tile([P, C], BF16, tag="h2")
            layernorm_tile(x_tok[:, nt, :], h_t, lnt2, stats2)
            transpose_into(h_t, h2_fm, nt, pst2, identb)

        if "o_fm" in tap_d:
            nc.sync.dma_start(
                tap_d["o_fm"].rearrange("(h p) n -> p h n", p=P), o_fm[:]
            )
        if "x1" in tap_d:
            nc.sync.dma_start(
                tap_d["x1"].rearrange("(nt p) c -> p nt c", p=P), x_tok[:]
            )
        if "h2_fm" in tap_d:
            nc.sync.dma_start(
                tap_d["h2_fm"].rearrange("(kt p) n -> p kt n", p=P), h2_fm[:]
            )

        stats2.release()
        lnt2.release()
        pst2.release()
        pops.release()

        wprojp.release()
        opool.release()

        # ---------- phase 8: MLP (bf16) ----------
        mlpc = tc.alloc_tile_pool(name="mlpc", bufs=1, side="left")
        bf2f = mlpc.tile([1, C], F32)
        nc.gpsimd.dma_start(
            bf2f[0:1, :], bass.AP(tensor=bf2_d, offset=0, ap=[[0, 1], [1, C]])
        )
        bf2s = mlpc.tile([1, C], BF16)
        nc.scalar.activation(
            out=bf2s[0:1, :], in_=bf2f[0:1, :],
            func=mybir.ActivationFunctionType.Copy,
        )
        ones1 = mlpc.tile([1, P], BF16)
        nc.vector.memset(ones1, 1.0)
        gpool = tc.alloc_tile_pool(name="gpool", bufs=3, side="left")
        outs = tc.alloc_tile_pool(name="outs", bufs=2, side="left")
        gps = tc.alloc_tile_pool(name="gps", bufs=3, space="PSUM")
        x2ps = tc.alloc_tile_pool(name="x2ps", bufs=2, space="PSUM")

        QW = 256  # token quarter width
        for q in range(4):
            pa = [x2ps.tile([P, 512], F32, tag="x2a", name=f"pa{q}_{jj}")
                  for jj in range(2)]
            pb = [x2ps.tile([P, 512], F32, tag="x2b", name=f"pb{q}_{jj}")
                  for jj in range(2)]
            for j in range(2):
                nc.tensor.matmul(
                    pa[j][:, 0:512], ones1[0:1, :], bf2s[0:1, 0:512],
                    start=True, stop=False,
                )
                nc.tensor.matmul(
                    pb[j][:, 0:256], ones1[0:1, :], bf2s[0:1, 512:768],
                    start=True, stop=False,
                )
            def emit_fc1(ff):
                pg = gps.tile([P, 512], F32, tag="G", name=f"pg_{q}_{ff}")
                for kt in range(KT):
                    nc.tensor.matmul(
                        pg[:, 0:QW],
                        wfc1[:, kt, ff * P:(ff + 1) * P],
                        h2_fm[:, kt, q * QW:(q + 1) * QW],
                        start=(kt == 0), stop=(kt == KT - 1),
                    )
                g_t = gpool.tile([P, QW], BF16, tag="g", name=f"g_{q}_{ff}")
                nc.scalar.activation(
                    out=g_t[:], in_=pg[:, 0:QW],
                    func=mybir.ActivationFunctionType.Gelu,
                    bias=bf1c[:, ff:ff + 1], scale=1.0,
                )
                return g_t

            def emit_fc2(ff, g_t):
                for j in range(2):
                    nc.tensor.matmul(
                        pa[j][:, 0:512],
                        g_t[:, j * P:(j + 1) * P],
                        wfc2[:, ff, 0:512],
                        start=False, stop=(ff == FFT - 1),
                    )
                    nc.tensor.matmul(
                        pb[j][:, 0:256],
                        g_t[:, j * P:(j + 1) * P],
                        wfc2[:, ff, 512:768],
                        start=False, stop=(ff == FFT - 1),
                    )

            g_prev = emit_fc1(0)
            for ff in range(FFT):
                if ff + 1 < FFT:
                    g_next = emit_fc1(ff + 1)
                else:
                    g_next = None
                emit_fc2(ff, g_prev)
                g_prev = g_next
            for j in range(2):
                nt = 2 * q + j
                o_t = outs.tile([P, C], F32, tag="y")
                nc.vector.tensor_add(
                    o_t[:, 0:512], pa[j][:, 0:512], x_tok[:, nt, 0:512]
                )
                nc.vector.tensor_add(
                    o_t[:, 512:768], pb[j][:, 0:256], x_tok[:, nt, 512:768]
                )
                nc.scalar.dma_start(y_d[nt * P:(nt + 1) * P, :], o_t[:])

        x2ps.release()
        gps.release()
        outs.release()
        gpool.release()
        mlpc.release()
        wfc2p.release()
        h2p.release()
        wfc1p.release()
        xpool.release()
        consts.release()

    nc.compile()
    return nc


def _f8(a):
    return np.ascontiguousarray(
        np.clip(a, -240.0, 240.0).astype(ml_dtypes.float8_e4m3)
    )


def _prep_inputs(inputs):
    """Host-side prep (exact refactoring of LN gains/biases into weights)."""
    f = lambda k: np.asarray(inputs[k], dtype=np.float32)
    x = f("x")
    w_qkv, w_proj, w_fc1, w_fc2 = f("w_qkv"), f("w_proj"), f("w_fc1"), f("w_fc2")
    ln1_g, ln1_b, ln2_g, ln2_b = f("ln1_g"), f("ln1_b"), f("ln2_g"), f("ln2_b")
    b_proj, b_fc1, b_fc2 = f("b_proj"), f("b_fc1"), f("b_fc2")

    bf = ml_dtypes.bfloat16
    w_qkv_e = ln1_g[:, None] * w_qkv
    qkv_bias = ln1_b @ w_qkv  # [2304]
    qk_bias = np.zeros((P, 2 * H), dtype=np.float32)
    for which in range(2):
        for h in range(H):
            qk_bias[0:DH, which * H + h] = qkv_bias[
                which * C + h * DH: which * C + (h + 1) * DH
            ]
    vb = qkv_bias[2 * C: 3 * C]  # v bias passes through softmax additively
    b_proj_e = b_proj + vb @ w_proj
    # head-aligned w_proj rows: block h rows 1..97 (row 0 pairs with colsum
    # row); row 97 of block 0 carries the proj bias (pairs with o_fm==1 row)
    w_proj_p = np.zeros((H * P, C), dtype=np.float32)
    for h in range(H):
        w_proj_p[h * P + 1: h * P + 1 + DH, :] = w_proj[h * DH:(h + 1) * DH, :]
    w_proj_p[DH + 1, :] = b_proj_e
    w_fc1_e = ln2_g[:, None] * w_fc1
    b_fc1_e = b_fc1 + ln2_b @ w_fc1

    common = {
        "w_qkv_e": _f8(w_qkv_e),
        "w_proj_p": _f8(w_proj_p),
        "w_fc1_e": np.ascontiguousarray(w_fc1_e.astype(bf)),
        "w_fc2": np.ascontiguousarray(w_fc2.astype(bf)),
        "qk_bias": qk_bias,
        "b_fc1_e": b_fc1_e,
        "b_fc2": b_fc2,
    }
    return [dict(common, x=np.ascontiguousarray(x[i])) for i in range(8)]


def kernel(**inputs):
    if "nc" not in _CACHED:
        _CACHED["nc"] = build()
    nc = _CACHED["nc"]
    in_maps = _prep_inputs(inputs)
    res = run_bass_kernel_spmd(nc, in_maps, core_ids=list(range(8)))
    out = np.stack([res.results[i]["y"] for i in range(8)], axis=0)
    return out.astype(np.float32)
